# revision 14
# baseline (speedup 1.0000x reference)
"""Multi-head self-attention (causal) Trainium2 Bass kernel, 8-core SPMD.

Problem: B=2, S=2048, D_IN=1024, D_OUT=1024, H=16 heads (hd=64), causal
softmax, out-proj with bias.

Sharding: core c -> (batch b = c // 4, head-group g = c % 4). Each core
computes 4 heads of one batch: data-parallel over b, tensor-parallel over
heads (W_q/W_k/W_v column shards, W_o row shard). Host sums the 4 partial
out-proj results per batch and adds b_o.

On-core layout is fully transposed (feature-major):
  xT   [D_IN, S]                (host pre-transposes x[b])
  Qt,Kt[hd*4, S]  = W^T @ xT    (pair-major: 2 sbuf tiles of [128, S])
  V    [S, hd*4]  (normal orientation, ones column appended per head)
  St   [k, q] scores transposed; Pt = exp(St/8) in bf16
  ctxT [hd*4, S] with softmax denominator from the ones column
  outT [D_OUT, S] partial = Wo_shard^T @ ctxT  (host transposes back)

All matmul operands bf16 (fp32 PSUM accumulate); softmax in fp32.
Scores for a head pair are packed: head0 on PE row-groups 0-1, head1 on
2-3 (concurrent matmuls), psum tiles merged so one ACT exp call covers
both heads of a k-block.
"""

import numpy as np
import ml_dtypes

import concourse.bass as bass
import concourse.bacc as bacc
import concourse.tile as tile
import concourse.mybir as mybir
from concourse.bass_utils import run_bass_kernel_spmd

N_CORES = 8
B, S, D_IN, D_OUT, H = 2, 2048, 1024, 1024, 16
H_LOC = 4  # heads per core
HD = 64
DH = H_LOC * HD  # 256 = d_out shard per core
KI = D_IN // 128  # 8 contraction chunks
NQ = S // 512  # 4 seq chunks of 512
NB = S // 128  # 16 seq blocks of 128
SCALE = 1.0 / np.sqrt(np.float32(HD))  # 0.125

BF16 = mybir.dt.bfloat16
F32 = mybir.dt.float32
EXP = mybir.ActivationFunctionType.Exp


def _pair_view(ap2d):
    """[128, 1024] tile -> [128, 2, 512] (head-major) view."""
    return ap2d.rearrange("p (h q) -> p h q", h=2)


def _build_body(nc, tc, xT_d, wq_d, wk_d, wv_d, wo_d, outT_d):
    from contextlib import ExitStack

    ctx = ExitStack()
    const = ctx.enter_context(tc.tile_pool(name="const", bufs=1))
    # PSUM: sc [128,1024]x2 = 4 banks; ctx [128,1024]x1 = 2; qk [128,512]x2 = 2
    sc_ps = ctx.enter_context(tc.tile_pool(name="sc_ps", bufs=2, space="PSUM"))
    ctx_ps = ctx.enter_context(tc.tile_pool(name="ctx_ps", bufs=1, space="PSUM"))
    qk_ps = ctx.enter_context(tc.tile_pool(name="qk_ps", bufs=2, space="PSUM"))
    pt_pool = ctx.enter_context(tc.tile_pool(name="pt", bufs=6))
    ob_pool = ctx.enter_context(tc.tile_pool(name="ob", bufs=3))
    r_pool = ctx.enter_context(tc.tile_pool(name="r", bufs=4))
    r64_pool = ctx.enter_context(tc.tile_pool(name="r64", bufs=4))

    # ---- resident SBUF tensors ----
    xT_s = const.tile([128, KI, S], BF16)
    wq_s = const.tile([128, KI, DH], BF16)
    wk_s = const.tile([128, KI, DH], BF16)
    wv_s = const.tile([128, KI, DH], BF16)
    wo_s = const.tile([128, 2, D_OUT], BF16)
    qt_s = const.tile([128, 2, S], BF16)  # [64*sub + hd, pair, q]
    kt_s = const.tile([128, 2, S], BF16)
    v_s = const.tile([128, NB, H_LOC, 2 * HD], BF16)  # cols HD.. = ones x64
    ctxT_s = const.tile([128, 2, S], BF16)
    obacc = const.tile([128, 8, S], BF16)  # pair-0 out-proj partial
    triL = const.tile([128, 128], BF16)  # L[c,k] = 1 iff k > c
    negdiag = const.tile([128, 128], BF16)  # -1e9 * I

    # ---- input DMAs (single large transfers; weights on the scalar queue) ----
    nc.sync.dma_start(
        out=xT_s, in_=xT_d.rearrange("(c p) s -> p c s", p=128)
    )
    nc.scalar.dma_start(
        out=wq_s, in_=wq_d.rearrange("(c p) s -> p c s", p=128)
    )
    nc.scalar.dma_start(
        out=wk_s, in_=wk_d.rearrange("(c p) s -> p c s", p=128)
    )
    nc.scalar.dma_start(
        out=wv_s, in_=wv_d.rearrange("(c p) s -> p c s", p=128)
    )
    nc.scalar.dma_start(
        out=wo_s, in_=wo_d.rearrange("(c p) s -> p c s", p=128)
    )

    # ---- constants for the PE-side causal mask ----
    # spv[k,q] += sum_c L[c,k] * negdiag[c,q] = -1e9 * (k > q)
    nc.vector.memset(triL, 1.0)
    nc.gpsimd.affine_select(  # keep 1 where k >= c+1, else 0
        out=triL, in_=triL, compare_op=mybir.AluOpType.is_ge,
        fill=0.0, base=-1, pattern=[[1, 128]], channel_multiplier=-1,
    )
    nc.vector.memset(negdiag, -1e9)
    nc.gpsimd.affine_select(  # keep where k >= c
        out=negdiag, in_=negdiag, compare_op=mybir.AluOpType.is_ge,
        fill=0.0, base=0, pattern=[[1, 128]], channel_multiplier=-1,
    )
    nc.gpsimd.affine_select(  # keep where k <= c  -> only diagonal survives
        out=negdiag, in_=negdiag, compare_op=mybir.AluOpType.is_ge,
        fill=0.0, base=0, pattern=[[-1, 128]], channel_multiplier=1,
    )
    # ones columns of V: ctx matmul rows 64..127 = replicated denominator
    nc.vector.memset(v_s[:, :, :, HD:], 1.0)

    def qk_proj(pair, s4):
        for w_s, dst in ((wq_s, qt_s), (wk_s, kt_s)):
            ps = qk_ps.tile([128, 512], F32, tag="qk", name="psqk")
            for ki in range(KI):
                nc.tensor.matmul(
                    ps,
                    lhsT=w_s[:, ki, 128 * pair : 128 * (pair + 1)],
                    rhs=xT_s[:, ki, 512 * s4 : 512 * (s4 + 1)],
                    start=(ki == 0),
                    stop=(ki == KI - 1),
                )
            nc.vector.tensor_copy(
                out=dst[:, pair, 512 * s4 : 512 * (s4 + 1)], in_=ps
            )

    def v_proj(sb):
        ps = qk_ps.tile([128, 256], F32, tag="qk", name="psv")
        for ki in range(KI):
            nc.tensor.matmul(
                ps,
                lhsT=xT_s[:, ki, 128 * sb : 128 * (sb + 1)],
                rhs=wv_s[:, ki, :],
                start=(ki == 0),
                stop=(ki == KI - 1),
            )
        nc.vector.tensor_copy(
            out=v_s[:, sb, :, 0:HD],
            in_=ps.rearrange("p (h d) -> p h d", h=H_LOC),
        )

    # pair-0 Q/K and V interleaved per seq chunk: attention can begin after
    # the first chunk of each is done
    for s4 in range(NQ):
        qk_proj(0, s4)
        for sb in range(4 * s4, 4 * s4 + 4):
            v_proj(sb)

    def outproj_partial(m, s4):
        op = qk_ps.tile([128, 512], F32, tag="qk", name="psop")
        nc.tensor.matmul(
            op,
            lhsT=wo_s[:, 0, 128 * m : 128 * (m + 1)],
            rhs=ctxT_s[:, 0, 512 * s4 : 512 * (s4 + 1)],
            start=True,
            stop=True,
        )
        nc.vector.tensor_copy(
            out=obacc[:, m, 512 * s4 : 512 * (s4 + 1)], in_=op
        )

    for pair in range(2):
        if pair == 1:
            for s4 in range(NQ):
                qk_proj(1, s4)
            # pair-0 partial out-proj: fills PE/DVE slack during pair-1 attn
            for m in range(8):
                for s4 in range(NQ):
                    outproj_partial(m, s4)

        # ---- attention for this pair (q chunks of 512, transposed) ----
        for jj in range(NQ):
            q0 = 512 * jj
            nkb = 4 * (jj + 1)
            cp = ctx_ps.tile([128, 1024], F32, tag="ctx", name="cp")
            cpv = _pair_view(cp)  # [65, 2, 512]
            for kb in range(nkb):
                d = kb - 4 * jj
                off = max(0, 128 * d)
                sp = sc_ps.tile([128, 1024], F32, tag="sc", name="sp")
                spv = _pair_view(sp)
                # paired scores matmuls (row groups 0-1 / 2-3 concurrent)
                for i in range(2):
                    nc.tensor.matmul(
                        spv[:, i, off:512],
                        lhsT=kt_s[
                            64 * i : 64 * i + 64, pair, 128 * kb : 128 * (kb + 1)
                        ],
                        rhs=qt_s[64 * i : 64 * i + 64, pair, q0 + off : q0 + 512],
                        start=True,
                        stop=True,
                    )
                if d >= 0:  # diagonal block: accumulate -1e9 upper triangle
                    for i in range(2):
                        nc.tensor.matmul(
                            spv[:, i, off : off + 128],
                            lhsT=triL,
                            rhs=negdiag,
                            start=False,
                            stop=True,
                            skip_group_check=True,
                        )
                pt = pt_pool.tile([128, 1024], BF16, tag="pt")
                ptv = _pair_view(pt)
                nc.scalar.activation(
                    out=ptv[:, :, off:512],
                    in_=spv[:, :, off:512],
                    func=EXP,
                    scale=float(SCALE),
                )
                for i in range(2):
                    h = 2 * pair + i
                    nc.tensor.matmul(
                        cpv[:, i, off:512],
                        lhsT=v_s[:, kb, h, :],
                        rhs=ptv[:, i, off:512],
                        start=(kb == 0),
                        stop=(kb == nkb - 1),
                    )  # rows 0-63: ctx; rows 64-127: D replicated
            # normalize: rows 0-63 ctx, rows 64-127 = D already broadcast
            d0 = r_pool.tile([64, 1024], F32, tag="d0")
            nc.vector.tensor_copy(out=d0, in_=cp[HD:, :])
            r64 = r64_pool.tile([64, 1024], F32, tag="r64")
            nc.vector.reciprocal_approx_fast(out=r64, in_=d0)
            for i in range(2):
                nc.vector.tensor_mul(
                    out=ctxT_s[64 * i : 64 * i + 64, pair, q0 : q0 + 512],
                    in0=cpv[0:HD, i, :],
                    in1=r64[:, 512 * i : 512 * (i + 1)],
                )

    # ---- final out-proj: pair-1 chunk + pair-0 partial (bf16 out) ----
    for m in range(8):
        for s2 in range(2):  # 1024-wide q chunks
            op = sc_ps.tile([128, 1024], F32, tag="sc", name="pso")
            for half in range(2):
                qlo = 1024 * s2 + 512 * half
                nc.tensor.matmul(
                    op[:, 512 * half : 512 * (half + 1)],
                    lhsT=wo_s[:, 1, 128 * m : 128 * (m + 1)],
                    rhs=ctxT_s[:, 1, qlo : qlo + 512],
                    start=True,
                    stop=True,
                )
            ob = ob_pool.tile([128, 1024], BF16, tag="ob")
            nc.vector.tensor_add(
                out=ob, in0=op, in1=obacc[:, m, 1024 * s2 : 1024 * (s2 + 1)]
            )
            nc.sync.dma_start(
                out=outT_d[128 * m : 128 * (m + 1), 1024 * s2 : 1024 * (s2 + 1)],
                in_=ob,
            )

    ctx.close()


_CACHED_NC = None


def _get_nc():
    global _CACHED_NC
    if _CACHED_NC is not None:
        return _CACHED_NC
    nc = bacc.Bacc(
        "TRN2", target_bir_lowering=False, debug=False, num_devices=N_CORES
    )
    xT_d = nc.dram_tensor("xT", [D_IN, S], BF16, kind="ExternalInput").ap()
    wq_d = nc.dram_tensor("wq", [D_IN, DH], BF16, kind="ExternalInput").ap()
    wk_d = nc.dram_tensor("wk", [D_IN, DH], BF16, kind="ExternalInput").ap()
    wv_d = nc.dram_tensor("wv", [D_IN, DH], BF16, kind="ExternalInput").ap()
    wo_d = nc.dram_tensor("wo", [DH, D_OUT], BF16, kind="ExternalInput").ap()
    outT_d = nc.dram_tensor("outT", [D_OUT, S], BF16, kind="ExternalOutput").ap()
    with tile.TileContext(nc) as tc:
        _build_body(nc, tc, xT_d, wq_d, wk_d, wv_d, wo_d, outT_d)
    nc.compile()
    _CACHED_NC = nc
    return nc


def _make_in_maps(x, W_q, W_k, W_v, W_o):
    bf = ml_dtypes.bfloat16
    in_maps = []
    xT = [np.ascontiguousarray(x[b].T).astype(bf) for b in range(B)]
    for c in range(N_CORES):
        b, g = c // 4, c % 4
        sl = slice(DH * g, DH * (g + 1))
        in_maps.append(
            {
                "xT": xT[b],
                "wq": np.ascontiguousarray(W_q[:, sl]).astype(bf),
                "wk": np.ascontiguousarray(W_k[:, sl]).astype(bf),
                "wv": np.ascontiguousarray(W_v[:, sl]).astype(bf),
                "wo": np.ascontiguousarray(W_o[sl, :]).astype(bf),
            }
        )
    return in_maps


def run_cores(x, W_q, W_k, W_v, W_o, **spmd_kwargs):
    """Compile (cached), run on 8 cores, return raw results object."""
    nc = _get_nc()
    in_maps = _make_in_maps(x, W_q, W_k, W_v, W_o)
    return run_bass_kernel_spmd(
        nc, in_maps, core_ids=list(range(N_CORES)), **spmd_kwargs
    )


def gather(results, b_o):
    out = np.empty((B, S, D_OUT), np.float32)
    for b in range(B):
        acc = results[4 * b]["outT"].astype(np.float32).copy()
        for g in range(1, 4):
            acc += results[4 * b + g]["outT"]
        out[b] = acc.T + b_o.astype(np.float32)[None, :]
    return out


def kernel(x, W_q, W_k, W_v, W_o, b_o):
    x = np.asarray(x)
    res = run_cores(
        x, np.asarray(W_q), np.asarray(W_k), np.asarray(W_v), np.asarray(W_o)
    )
    return gather(res.results, np.asarray(b_o))


# revision 18
# speedup vs baseline: 1.1136x; 1.1136x over previous
"""Multi-head self-attention (causal) Trainium2 Bass kernel, 8-core SPMD.

Problem: B=2, S=2048, D_IN=1024, D_OUT=1024, H=16 heads (hd=64), causal
softmax, out-proj with bias.

Sharding: core c -> (batch b = c // 4, head-group g = c % 4). Each core
computes 4 heads of one batch: data-parallel over b, tensor-parallel over
heads (W_q/W_k/W_v column shards, W_o row shard). Host sums the 4 partial
out-proj results per batch and adds b_o.

On-core layout is fully transposed (feature-major):
  xT   [D_IN, S]                (host pre-transposes x[b])
  Qt,Kt[hd*4, S]  = W^T @ xT    (pair-major: 2 sbuf tiles of [128, S])
  V    [S, hd*4]  (normal orientation, ones column appended per head)
  St   [k, q] scores transposed; Pt = exp(St/8) in bf16
  ctxT [hd*4, S] with softmax denominator from the ones column
  outT [D_OUT, S] partial = Wo_shard^T @ ctxT  (host transposes back)

All matmul operands bf16 (fp32 PSUM accumulate); softmax in fp32.
Scores for a head pair are packed: head0 on PE row-groups 0-1, head1 on
2-3 (concurrent matmuls), psum tiles merged so one ACT exp call covers
both heads of a k-block.
"""

import numpy as np
import ml_dtypes

import concourse.bass as bass
import concourse.bacc as bacc
import concourse.tile as tile
import concourse.mybir as mybir
from concourse.bass_utils import run_bass_kernel_spmd

N_CORES = 8
B, S, D_IN, D_OUT, H = 2, 2048, 1024, 1024, 16
H_LOC = 4  # heads per core
HD = 64
DH = H_LOC * HD  # 256 = d_out shard per core
KI = D_IN // 128  # 8 contraction chunks
NQ = S // 512  # 4 seq chunks of 512
NB = S // 128  # 16 seq blocks of 128
SCALE = 1.0 / np.sqrt(np.float32(HD))  # 0.125

BF16 = mybir.dt.bfloat16
F32 = mybir.dt.float32
EXP = mybir.ActivationFunctionType.Exp


def _pair_view(ap2d):
    """[128, 1024] tile -> [128, 2, 512] (head-major) view."""
    return ap2d.rearrange("p (h q) -> p h q", h=2)


def _build_body(nc, tc, xT_d, wq_d, wk_d, wv_d, wo_d, outT_d):
    from contextlib import ExitStack

    ctx = ExitStack()
    const = ctx.enter_context(tc.tile_pool(name="const", bufs=1))
    # PSUM: sc [128,1024]x2 = 4 banks; ctx [128,1024]x1 = 2; qk [128,512]x2 = 2
    sc_ps = ctx.enter_context(tc.tile_pool(name="sc_ps", bufs=2, space="PSUM"))
    ctx_ps = ctx.enter_context(tc.tile_pool(name="ctx_ps", bufs=1, space="PSUM"))
    qk_ps = ctx.enter_context(tc.tile_pool(name="qk_ps", bufs=2, space="PSUM"))
    pt_pool = ctx.enter_context(tc.tile_pool(name="pt", bufs=6))
    ob_pool = ctx.enter_context(tc.tile_pool(name="ob", bufs=3))
    r_pool = ctx.enter_context(tc.tile_pool(name="r", bufs=4))
    r64_pool = ctx.enter_context(tc.tile_pool(name="r64", bufs=4))

    # ---- resident SBUF tensors ----
    xT_s = const.tile([128, KI, S], BF16)
    wq_s = const.tile([128, KI, DH], BF16)
    wk_s = const.tile([128, KI, DH], BF16)
    wv_s = const.tile([128, KI, DH], BF16)
    wo_s = const.tile([128, 2, D_OUT], BF16)
    qt_s = const.tile([128, 2, S], BF16)  # [64*sub + hd, pair, q]
    kt_s = const.tile([128, 2, S], BF16)
    v_s = const.tile([128, NB, H_LOC, 2 * HD], BF16)  # cols HD.. = ones x64
    ctxT_s = const.tile([128, 2, S], BF16)
    triL = const.tile([128, 128], BF16)  # L[c,k] = 1 iff k > c
    negdiag = const.tile([128, 128], BF16)  # -1e9 * I

    # ---- input DMAs (single large transfers; weights on the scalar queue) ----
    nc.sync.dma_start(
        out=xT_s, in_=xT_d.rearrange("(c p) s -> p c s", p=128)
    )
    nc.scalar.dma_start(
        out=wq_s, in_=wq_d.rearrange("(c p) s -> p c s", p=128)
    )
    nc.scalar.dma_start(
        out=wk_s, in_=wk_d.rearrange("(c p) s -> p c s", p=128)
    )
    nc.scalar.dma_start(
        out=wv_s, in_=wv_d.rearrange("(c p) s -> p c s", p=128)
    )
    nc.scalar.dma_start(
        out=wo_s, in_=wo_d.rearrange("(c p) s -> p c s", p=128)
    )

    # ---- constants for the PE-side causal mask ----
    # spv[k,q] += sum_c L[c,k] * negdiag[c,q] = -1e9 * (k > q)
    nc.vector.memset(triL, 1.0)
    nc.gpsimd.affine_select(  # keep 1 where k >= c+1, else 0
        out=triL, in_=triL, compare_op=mybir.AluOpType.is_ge,
        fill=0.0, base=-1, pattern=[[1, 128]], channel_multiplier=-1,
    )
    nc.vector.memset(negdiag, -1e9)
    nc.gpsimd.affine_select(  # keep where k >= c
        out=negdiag, in_=negdiag, compare_op=mybir.AluOpType.is_ge,
        fill=0.0, base=0, pattern=[[1, 128]], channel_multiplier=-1,
    )
    nc.gpsimd.affine_select(  # keep where k <= c  -> only diagonal survives
        out=negdiag, in_=negdiag, compare_op=mybir.AluOpType.is_ge,
        fill=0.0, base=0, pattern=[[-1, 128]], channel_multiplier=1,
    )
    # ones columns of V: ctx matmul rows 64..127 = replicated denominator
    nc.vector.memset(v_s[:, :, :, HD:], 1.0)

    def qk_proj(pair, s4):
        for w_s, dst in ((wq_s, qt_s), (wk_s, kt_s)):
            ps = qk_ps.tile([128, 512], F32, tag="qk", name="psqk")
            for ki in range(KI):
                nc.tensor.matmul(
                    ps,
                    lhsT=w_s[:, ki, 128 * pair : 128 * (pair + 1)],
                    rhs=xT_s[:, ki, 512 * s4 : 512 * (s4 + 1)],
                    start=(ki == 0),
                    stop=(ki == KI - 1),
                )
            nc.vector.tensor_copy(
                out=dst[:, pair, 512 * s4 : 512 * (s4 + 1)], in_=ps
            )

    def v_proj(sb):
        ps = qk_ps.tile([128, 256], F32, tag="qk", name="psv")
        for ki in range(KI):
            nc.tensor.matmul(
                ps,
                lhsT=xT_s[:, ki, 128 * sb : 128 * (sb + 1)],
                rhs=wv_s[:, ki, :],
                start=(ki == 0),
                stop=(ki == KI - 1),
            )
        nc.vector.tensor_copy(
            out=v_s[:, sb, :, 0:HD],
            in_=ps.rearrange("p (h d) -> p h d", h=H_LOC),
        )

    # pair-0 Q/K and V interleaved per seq chunk: attention can begin after
    # the first chunk of each is done
    for s4 in range(NQ):
        qk_proj(0, s4)
        for sb in range(4 * s4, 4 * s4 + 4):
            v_proj(sb)

    def outproj(m, s4):
        """Full out-proj tile (both pair chunks) -> bf16 -> DRAM."""
        op = qk_ps.tile([128, 512], F32, tag="qk", name="psop")
        for c in range(2):
            nc.tensor.matmul(
                op,
                lhsT=wo_s[:, c, 128 * m : 128 * (m + 1)],
                rhs=ctxT_s[:, c, 512 * s4 : 512 * (s4 + 1)],
                start=(c == 0),
                stop=(c == 1),
            )
        ob = ob_pool.tile([128, 512], BF16, tag="ob")
        nc.vector.tensor_copy(out=ob, in_=op)
        nc.sync.dma_start(
            out=outT_d[128 * m : 128 * (m + 1), 512 * s4 : 512 * (s4 + 1)],
            in_=ob,
        )

    for pair in range(2):
        # ---- attention for this pair (q chunks of 512, transposed) ----
        for jj in range(NQ):
            q0 = 512 * jj
            nkb = 4 * (jj + 1)
            cp = ctx_ps.tile([128, 1024], F32, tag="ctx", name="cp")
            cpv = _pair_view(cp)  # [65, 2, 512]
            for kb in range(nkb):
                d = kb - 4 * jj
                off = max(0, 128 * d)
                sp = sc_ps.tile([128, 1024], F32, tag="sc", name="sp")
                spv = _pair_view(sp)
                # paired scores matmuls (row groups 0-1 / 2-3 concurrent)
                for i in range(2):
                    nc.tensor.matmul(
                        spv[:, i, off:512],
                        lhsT=kt_s[
                            64 * i : 64 * i + 64, pair, 128 * kb : 128 * (kb + 1)
                        ],
                        rhs=qt_s[64 * i : 64 * i + 64, pair, q0 + off : q0 + 512],
                        start=True,
                        stop=True,
                    )
                if d >= 0:  # diagonal block: accumulate -1e9 upper triangle
                    for i in range(2):
                        nc.tensor.matmul(
                            spv[:, i, off : off + 128],
                            lhsT=triL,
                            rhs=negdiag,
                            start=False,
                            stop=True,
                            skip_group_check=True,
                        )
                pt = pt_pool.tile([128, 1024], BF16, tag="pt")
                ptv = _pair_view(pt)
                nc.scalar.activation(
                    out=ptv[:, :, off:512],
                    in_=spv[:, :, off:512],
                    func=EXP,
                    scale=float(SCALE),
                )
                for i in range(2):
                    h = 2 * pair + i
                    nc.tensor.matmul(
                        cpv[:, i, off:512],
                        lhsT=v_s[:, kb, h, :],
                        rhs=ptv[:, i, off:512],
                        start=(kb == 0),
                        stop=(kb == nkb - 1),
                    )  # rows 0-63: ctx; rows 64-127: D replicated
            # stage psum -> sbuf (releases cp); rows 0-63 ctx, rows 64-127 =
            # D broadcast. D staged to base partition 0: custom-DVE ops
            # (reciprocal_approx_fast) only work at partition offset 0.
            stage = r_pool.tile([64, 1024], F32, tag="stage")
            nc.vector.tensor_copy(out=stage, in_=cp[0:HD, :])
            std = r_pool.tile([64, 1024], F32, tag="std")
            nc.vector.tensor_copy(out=std, in_=cp[HD:, :])
            stv = stage.rearrange("p (h q) -> p h q", h=2)
            r64 = r64_pool.tile([64, 1024], F32, tag="r64")
            nc.vector.reciprocal_approx_fast(out=r64, in_=std)
            for i in range(2):
                nc.vector.tensor_mul(
                    out=ctxT_s[64 * i : 64 * i + 64, pair, q0 : q0 + 512],
                    in0=stv[0:HD, i, :],
                    in1=r64[:, 512 * i : 512 * (i + 1)],
                )
            if pair == 0:
                # pair-1 Q/K projection: fills PE slack during pair-0 attn
                qk_proj(1, jj)
            else:
                # out-proj for this seq chunk (both pair contributions ready)
                for m in range(8):
                    outproj(m, jj)

    ctx.close()


_CACHED_NC = None


def _get_nc():
    global _CACHED_NC
    if _CACHED_NC is not None:
        return _CACHED_NC
    nc = bacc.Bacc(
        "TRN2", target_bir_lowering=False, debug=False, num_devices=N_CORES
    )
    xT_d = nc.dram_tensor("xT", [D_IN, S], BF16, kind="ExternalInput").ap()
    wq_d = nc.dram_tensor("wq", [D_IN, DH], BF16, kind="ExternalInput").ap()
    wk_d = nc.dram_tensor("wk", [D_IN, DH], BF16, kind="ExternalInput").ap()
    wv_d = nc.dram_tensor("wv", [D_IN, DH], BF16, kind="ExternalInput").ap()
    wo_d = nc.dram_tensor("wo", [DH, D_OUT], BF16, kind="ExternalInput").ap()
    outT_d = nc.dram_tensor("outT", [D_OUT, S], BF16, kind="ExternalOutput").ap()
    with tile.TileContext(nc) as tc:
        _build_body(nc, tc, xT_d, wq_d, wk_d, wv_d, wo_d, outT_d)
    nc.compile()
    _CACHED_NC = nc
    return nc


def _make_in_maps(x, W_q, W_k, W_v, W_o):
    bf = ml_dtypes.bfloat16
    in_maps = []
    xT = [np.ascontiguousarray(x[b].T).astype(bf) for b in range(B)]
    for c in range(N_CORES):
        b, g = c // 4, c % 4
        sl = slice(DH * g, DH * (g + 1))
        in_maps.append(
            {
                "xT": xT[b],
                "wq": np.ascontiguousarray(W_q[:, sl]).astype(bf),
                "wk": np.ascontiguousarray(W_k[:, sl]).astype(bf),
                "wv": np.ascontiguousarray(W_v[:, sl]).astype(bf),
                "wo": np.ascontiguousarray(W_o[sl, :]).astype(bf),
            }
        )
    return in_maps


def run_cores(x, W_q, W_k, W_v, W_o, **spmd_kwargs):
    """Compile (cached), run on 8 cores, return raw results object."""
    nc = _get_nc()
    in_maps = _make_in_maps(x, W_q, W_k, W_v, W_o)
    return run_bass_kernel_spmd(
        nc, in_maps, core_ids=list(range(N_CORES)), **spmd_kwargs
    )


def gather(results, b_o):
    out = np.empty((B, S, D_OUT), np.float32)
    for b in range(B):
        acc = results[4 * b]["outT"].astype(np.float32).copy()
        for g in range(1, 4):
            acc += results[4 * b + g]["outT"]
        out[b] = acc.T + b_o.astype(np.float32)[None, :]
    return out


def kernel(x, W_q, W_k, W_v, W_o, b_o):
    x = np.asarray(x)
    res = run_cores(
        x, np.asarray(W_q), np.asarray(W_k), np.asarray(W_v), np.asarray(W_o)
    )
    return gather(res.results, np.asarray(b_o))


# revision 19
# speedup vs baseline: 1.1230x; 1.0085x over previous
"""Multi-head self-attention (causal) Trainium2 Bass kernel, 8-core SPMD.

Problem: B=2, S=2048, D_IN=1024, D_OUT=1024, H=16 heads (hd=64), causal
softmax, out-proj with bias.

Sharding: core c -> (batch b = c // 4, head-group g = c % 4). Each core
computes 4 heads of one batch: data-parallel over b, tensor-parallel over
heads (W_q/W_k/W_v column shards, W_o row shard). Host sums the 4 partial
out-proj results per batch and adds b_o.

On-core layout is fully transposed (feature-major):
  xT   [D_IN, S]                (host pre-transposes x[b])
  Qt,Kt[hd*4, S]  = W^T @ xT    (pair-major: 2 sbuf tiles of [128, S])
  V    [S, hd*4]  (normal orientation, ones column appended per head)
  St   [k, q] scores transposed; Pt = exp(St/8) in bf16
  ctxT [hd*4, S] with softmax denominator from the ones column
  outT [D_OUT, S] partial = Wo_shard^T @ ctxT  (host transposes back)

All matmul operands bf16 (fp32 PSUM accumulate); softmax in fp32.
Scores for a head pair are packed: head0 on PE row-groups 0-1, head1 on
2-3 (concurrent matmuls), psum tiles merged so one ACT exp call covers
both heads of a k-block.
"""

import numpy as np
import ml_dtypes

import concourse.bass as bass
import concourse.bacc as bacc
import concourse.tile as tile
import concourse.mybir as mybir
from concourse.bass_utils import run_bass_kernel_spmd

N_CORES = 8
B, S, D_IN, D_OUT, H = 2, 2048, 1024, 1024, 16
H_LOC = 4  # heads per core
HD = 64
DH = H_LOC * HD  # 256 = d_out shard per core
KI = D_IN // 128  # 8 contraction chunks
NQ = S // 512  # 4 seq chunks of 512
NB = S // 128  # 16 seq blocks of 128
SCALE = 1.0 / np.sqrt(np.float32(HD))  # 0.125

BF16 = mybir.dt.bfloat16
F32 = mybir.dt.float32
EXP = mybir.ActivationFunctionType.Exp


def _pair_view(ap2d):
    """[128, 1024] tile -> [128, 2, 512] (head-major) view."""
    return ap2d.rearrange("p (h q) -> p h q", h=2)


def _build_body(nc, tc, xT_d, wq_d, wk_d, wv_d, wo_d, outT_d):
    from contextlib import ExitStack

    ctx = ExitStack()
    const = ctx.enter_context(tc.tile_pool(name="const", bufs=1))
    # PSUM: sc [128,1024]x2 = 4 banks; ctx [128,1024]x1 = 2; qk [128,512]x2 = 2
    sc_ps = ctx.enter_context(tc.tile_pool(name="sc_ps", bufs=2, space="PSUM"))
    ctx_ps = ctx.enter_context(tc.tile_pool(name="ctx_ps", bufs=1, space="PSUM"))
    qk_ps = ctx.enter_context(tc.tile_pool(name="qk_ps", bufs=2, space="PSUM"))
    pt_pool = ctx.enter_context(tc.tile_pool(name="pt", bufs=6))
    ob_pool = ctx.enter_context(tc.tile_pool(name="ob", bufs=3))
    r_pool = ctx.enter_context(tc.tile_pool(name="r", bufs=4))
    r64_pool = ctx.enter_context(tc.tile_pool(name="r64", bufs=4))

    # ---- resident SBUF tensors ----
    xT_s = const.tile([128, KI, S], BF16)
    wq_s = const.tile([128, KI, DH], BF16)
    wk_s = const.tile([128, KI, DH], BF16)
    wv_s = const.tile([128, KI, DH], BF16)
    wo_s = const.tile([128, 2, D_OUT], BF16)
    qt_s = const.tile([128, 2, S], BF16)  # [64*sub + hd, pair, q]
    kt_s = const.tile([128, 2, S], BF16)
    v_s = const.tile([128, NB, H_LOC, 2 * HD], BF16)  # cols HD.. = ones x64
    ctxT_s = const.tile([128, 2, S], BF16)
    triL = const.tile([128, 128], BF16)  # L[c,k] = 1 iff k > c
    negdiag = const.tile([128, 128], BF16)  # -1e9 * I

    # ---- input DMAs (host pre-arranged to on-chip layout; contiguous) ----
    nc.scalar.dma_start(out=wq_s, in_=wq_d.rearrange("p (c s) -> p c s", c=KI))
    nc.scalar.dma_start(out=wk_s, in_=wk_d.rearrange("p (c s) -> p c s", c=KI))
    nc.sync.dma_start(out=xT_s, in_=xT_d.rearrange("p (c s) -> p c s", c=KI))
    nc.scalar.dma_start(out=wv_s, in_=wv_d.rearrange("p (c s) -> p c s", c=KI))
    nc.scalar.dma_start(out=wo_s, in_=wo_d.rearrange("p (c s) -> p c s", c=2))

    # ---- constants for the PE-side causal mask ----
    # spv[k,q] += sum_c L[c,k] * negdiag[c,q] = -1e9 * (k > q)
    nc.vector.memset(triL, 1.0)
    nc.gpsimd.affine_select(  # keep 1 where k >= c+1, else 0
        out=triL, in_=triL, compare_op=mybir.AluOpType.is_ge,
        fill=0.0, base=-1, pattern=[[1, 128]], channel_multiplier=-1,
    )
    nc.vector.memset(negdiag, -1e9)
    nc.gpsimd.affine_select(  # keep where k >= c
        out=negdiag, in_=negdiag, compare_op=mybir.AluOpType.is_ge,
        fill=0.0, base=0, pattern=[[1, 128]], channel_multiplier=-1,
    )
    nc.gpsimd.affine_select(  # keep where k <= c  -> only diagonal survives
        out=negdiag, in_=negdiag, compare_op=mybir.AluOpType.is_ge,
        fill=0.0, base=0, pattern=[[-1, 128]], channel_multiplier=1,
    )
    # ones columns of V: ctx matmul rows 64..127 = replicated denominator
    nc.vector.memset(v_s[:, :, :, HD:], 1.0)

    def qk_one(pair, s4, which):
        w_s, dst = ((wq_s, qt_s), (wk_s, kt_s))[which]
        ps = qk_ps.tile([128, 512], F32, tag="qk", name="psqk")
        for ki in range(KI):
            nc.tensor.matmul(
                ps,
                lhsT=w_s[:, ki, 128 * pair : 128 * (pair + 1)],
                rhs=xT_s[:, ki, 512 * s4 : 512 * (s4 + 1)],
                start=(ki == 0),
                stop=(ki == KI - 1),
            )
        nc.vector.tensor_copy(
            out=dst[:, pair, 512 * s4 : 512 * (s4 + 1)], in_=ps
        )

    def qk_proj(pair, s4):
        qk_one(pair, s4, 0)
        qk_one(pair, s4, 1)

    def v_proj(sb):
        ps = qk_ps.tile([128, 256], F32, tag="qk", name="psv")
        for ki in range(KI):
            nc.tensor.matmul(
                ps,
                lhsT=xT_s[:, ki, 128 * sb : 128 * (sb + 1)],
                rhs=wv_s[:, ki, :],
                start=(ki == 0),
                stop=(ki == KI - 1),
            )
        nc.vector.tensor_copy(
            out=v_s[:, sb, :, 0:HD],
            in_=ps.rearrange("p (h d) -> p h d", h=H_LOC),
        )

    # pair-0 Q/K and V interleaved per seq chunk: attention can begin after
    # the first chunk of each is done
    for s4 in range(NQ):
        qk_proj(0, s4)
        for sb in range(4 * s4, 4 * s4 + 4):
            v_proj(sb)

    def outproj(m, s4):
        """Full out-proj tile (both pair chunks) -> bf16 -> DRAM."""
        op = qk_ps.tile([128, 512], F32, tag="qk", name="psop")
        for c in range(2):
            nc.tensor.matmul(
                op,
                lhsT=wo_s[:, c, 128 * m : 128 * (m + 1)],
                rhs=ctxT_s[:, c, 512 * s4 : 512 * (s4 + 1)],
                start=(c == 0),
                stop=(c == 1),
            )
        ob = ob_pool.tile([128, 512], BF16, tag="ob")
        nc.vector.tensor_copy(out=ob, in_=op)
        nc.sync.dma_start(
            out=outT_d[128 * m : 128 * (m + 1), 512 * s4 : 512 * (s4 + 1)],
            in_=ob,
        )

    for pair in range(2):
        # ---- attention for this pair (q chunks of 512, transposed) ----
        for jj in range(NQ):
            q0 = 512 * jj
            nkb = 4 * (jj + 1)
            cp = ctx_ps.tile([128, 1024], F32, tag="ctx", name="cp")
            cpv = _pair_view(cp)  # [65, 2, 512]
            for kb in range(nkb):
                d = kb - 4 * jj
                off = max(0, 128 * d)
                sp = sc_ps.tile([128, 1024], F32, tag="sc", name="sp")
                spv = _pair_view(sp)
                # paired scores matmuls (row groups 0-1 / 2-3 concurrent)
                for i in range(2):
                    nc.tensor.matmul(
                        spv[:, i, off:512],
                        lhsT=kt_s[
                            64 * i : 64 * i + 64, pair, 128 * kb : 128 * (kb + 1)
                        ],
                        rhs=qt_s[64 * i : 64 * i + 64, pair, q0 + off : q0 + 512],
                        start=True,
                        stop=True,
                    )
                if d >= 0:  # diagonal block: accumulate -1e9 upper triangle
                    for i in range(2):
                        nc.tensor.matmul(
                            spv[:, i, off : off + 128],
                            lhsT=triL,
                            rhs=negdiag,
                            start=False,
                            stop=True,
                            skip_group_check=True,
                        )
                pt = pt_pool.tile([128, 1024], BF16, tag="pt")
                ptv = _pair_view(pt)
                nc.scalar.activation(
                    out=ptv[:, :, off:512],
                    in_=spv[:, :, off:512],
                    func=EXP,
                    scale=float(SCALE),
                )
                for i in range(2):
                    h = 2 * pair + i
                    nc.tensor.matmul(
                        cpv[:, i, off:512],
                        lhsT=v_s[:, kb, h, :],
                        rhs=ptv[:, i, off:512],
                        start=(kb == 0),
                        stop=(kb == nkb - 1),
                    )  # rows 0-63: ctx; rows 64-127: D replicated
                if pair == 0 and jj >= 1 and kb in (1, 3):
                    # pair-1 Q/K projection spread through pair-0 attention
                    qk_one(1, jj, kb // 2)
                if pair == 1 and jj >= 1 and kb < 8:
                    # out-proj of the previous seq chunk, spread across kbs
                    outproj(kb, jj - 1)
            # stage psum -> sbuf (releases cp); rows 0-63 ctx, rows 64-127 =
            # D broadcast. D staged to base partition 0: custom-DVE ops
            # (reciprocal_approx_fast) only work at partition offset 0.
            stage = r_pool.tile([64, 1024], F32, tag="stage")
            nc.vector.tensor_copy(out=stage, in_=cp[0:HD, :])
            std = r_pool.tile([64, 1024], F32, tag="std")
            nc.vector.tensor_copy(out=std, in_=cp[HD:, :])
            stv = stage.rearrange("p (h q) -> p h q", h=2)
            r64 = r64_pool.tile([64, 1024], F32, tag="r64")
            nc.vector.reciprocal_approx_fast(out=r64, in_=std)
            for i in range(2):
                nc.vector.tensor_mul(
                    out=ctxT_s[64 * i : 64 * i + 64, pair, q0 : q0 + 512],
                    in0=stv[0:HD, i, :],
                    in1=r64[:, 512 * i : 512 * (i + 1)],
                )
            if pair == 0 and jj == 0:
                qk_one(1, 0, 0)
                qk_one(1, 0, 1)

    # ---- tail: out-proj of the last seq chunk ----
    for m in range(8):
        outproj(m, NQ - 1)

    ctx.close()


_CACHED_NC = None


def _get_nc():
    global _CACHED_NC
    if _CACHED_NC is not None:
        return _CACHED_NC
    nc = bacc.Bacc(
        "TRN2", target_bir_lowering=False, debug=False, num_devices=N_CORES
    )
    xT_d = nc.dram_tensor("xT", [128, KI * S], BF16, kind="ExternalInput").ap()
    wq_d = nc.dram_tensor("wq", [128, KI * DH], BF16, kind="ExternalInput").ap()
    wk_d = nc.dram_tensor("wk", [128, KI * DH], BF16, kind="ExternalInput").ap()
    wv_d = nc.dram_tensor("wv", [128, KI * DH], BF16, kind="ExternalInput").ap()
    wo_d = nc.dram_tensor("wo", [128, 2 * D_OUT], BF16, kind="ExternalInput").ap()
    outT_d = nc.dram_tensor("outT", [D_OUT, S], BF16, kind="ExternalOutput").ap()
    with tile.TileContext(nc) as tc:
        _build_body(nc, tc, xT_d, wq_d, wk_d, wv_d, wo_d, outT_d)
    nc.compile()
    _CACHED_NC = nc
    return nc


def _chunked(a):
    """[C*128, N] -> [128, C*N] (partition-major chunks, on-chip layout)."""
    c = a.shape[0] // 128
    return np.ascontiguousarray(
        a.reshape(c, 128, a.shape[1]).transpose(1, 0, 2).reshape(128, -1)
    )


def _make_in_maps(x, W_q, W_k, W_v, W_o):
    bf = ml_dtypes.bfloat16
    in_maps = []
    xT = [_chunked(np.ascontiguousarray(x[b].T)).astype(bf) for b in range(B)]
    for c in range(N_CORES):
        b, g = c // 4, c % 4
        sl = slice(DH * g, DH * (g + 1))
        in_maps.append(
            {
                "xT": xT[b],
                "wq": _chunked(np.ascontiguousarray(W_q[:, sl])).astype(bf),
                "wk": _chunked(np.ascontiguousarray(W_k[:, sl])).astype(bf),
                "wv": _chunked(np.ascontiguousarray(W_v[:, sl])).astype(bf),
                "wo": _chunked(np.ascontiguousarray(W_o[sl, :])).astype(bf),
            }
        )
    return in_maps


def run_cores(x, W_q, W_k, W_v, W_o, **spmd_kwargs):
    """Compile (cached), run on 8 cores, return raw results object."""
    nc = _get_nc()
    in_maps = _make_in_maps(x, W_q, W_k, W_v, W_o)
    return run_bass_kernel_spmd(
        nc, in_maps, core_ids=list(range(N_CORES)), **spmd_kwargs
    )


def gather(results, b_o):
    out = np.empty((B, S, D_OUT), np.float32)
    for b in range(B):
        acc = results[4 * b]["outT"].astype(np.float32).copy()
        for g in range(1, 4):
            acc += results[4 * b + g]["outT"]
        out[b] = acc.T + b_o.astype(np.float32)[None, :]
    return out


def kernel(x, W_q, W_k, W_v, W_o, b_o):
    x = np.asarray(x)
    res = run_cores(
        x, np.asarray(W_q), np.asarray(W_k), np.asarray(W_v), np.asarray(W_o)
    )
    return gather(res.results, np.asarray(b_o))


# revision 20
# speedup vs baseline: 1.1618x; 1.0346x over previous
"""Multi-head self-attention (causal) Trainium2 Bass kernel, 8-core SPMD.

Problem: B=2, S=2048, D_IN=1024, D_OUT=1024, H=16 heads (hd=64), causal
softmax, out-proj with bias.

Sharding: core c -> (batch b = c // 4, head-group g = c % 4). Each core
computes 4 heads of one batch: data-parallel over b, tensor-parallel over
heads (W_q/W_k/W_v column shards, W_o row shard). Host sums the 4 partial
out-proj results per batch and adds b_o.

On-core layout is fully transposed (feature-major):
  xT   [D_IN, S]                (host pre-transposes x[b])
  Qt,Kt[hd*4, S]  = W^T @ xT    (pair-major: 2 sbuf tiles of [128, S])
  V    [S, hd*4]  (normal orientation, ones column appended per head)
  St   [k, q] scores transposed; Pt = exp(St/8) in bf16
  ctxT [hd*4, S] with softmax denominator from the ones column
  outT [D_OUT, S] partial = Wo_shard^T @ ctxT  (host transposes back)

All matmul operands bf16 (fp32 PSUM accumulate); softmax in fp32.
Scores for a head pair are packed: head0 on PE row-groups 0-1, head1 on
2-3 (concurrent matmuls), psum tiles merged so one ACT exp call covers
both heads of a k-block.
"""

import numpy as np
import ml_dtypes

import concourse.bass as bass
import concourse.bacc as bacc
import concourse.tile as tile
import concourse.mybir as mybir
from concourse.bass_utils import run_bass_kernel_spmd

N_CORES = 8
B, S, D_IN, D_OUT, H = 2, 2048, 1024, 1024, 16
H_LOC = 4  # heads per core
HD = 64
DH = H_LOC * HD  # 256 = d_out shard per core
KI = D_IN // 128  # 8 contraction chunks
NQ = S // 512  # 4 seq chunks of 512
NB = S // 128  # 16 seq blocks of 128
SCALE = 1.0 / np.sqrt(np.float32(HD))  # 0.125

BF16 = mybir.dt.bfloat16
F32 = mybir.dt.float32
EXP = mybir.ActivationFunctionType.Exp


def _pair_view(ap2d):
    """[128, 1024] tile -> [128, 2, 512] (head-major) view."""
    return ap2d.rearrange("p (h q) -> p h q", h=2)


def _build_body(nc, tc, xT_d, wq_d, wk_d, wv_d, wo_d, outT_d):
    from contextlib import ExitStack

    ctx = ExitStack()
    const = ctx.enter_context(tc.tile_pool(name="const", bufs=1))
    # PSUM: sc [128,1024]x2 = 4 banks; ctx [128,1024]x1 = 2; qk [128,512]x2 = 2
    sc_ps = ctx.enter_context(tc.tile_pool(name="sc_ps", bufs=2, space="PSUM"))
    ctx_ps = ctx.enter_context(tc.tile_pool(name="ctx_ps", bufs=1, space="PSUM"))
    qk_ps = ctx.enter_context(tc.tile_pool(name="qk_ps", bufs=2, space="PSUM"))
    pt_pool = ctx.enter_context(tc.tile_pool(name="pt", bufs=6))
    ob_pool = ctx.enter_context(tc.tile_pool(name="ob", bufs=3))
    r_pool = ctx.enter_context(tc.tile_pool(name="r", bufs=4))
    r64_pool = ctx.enter_context(tc.tile_pool(name="r64", bufs=4))

    # ---- resident SBUF tensors ----
    xT_s = const.tile([128, KI, S], BF16)
    wq_s = const.tile([128, KI, DH], BF16)
    wk_s = const.tile([128, KI, DH], BF16)
    wv_s = const.tile([128, KI, DH], BF16)
    wo_s = const.tile([128, 2, D_OUT], BF16)
    qt_s = const.tile([128, 2, S], BF16)  # [64*sub + hd, pair, q]
    kt_s = const.tile([128, 2, S], BF16)
    v_s = const.tile([128, NB, H_LOC, 2 * HD], BF16)  # cols HD.. = ones x64
    ctxT_s = const.tile([128, 2, S], BF16)
    triL = const.tile([128, 128], BF16)  # L[c,k] = 1 iff k > c
    negdiag = const.tile([128, 128], BF16)  # -1e9 * I

    # ---- input DMAs (host pre-arranged to on-chip layout; contiguous) ----
    nc.scalar.dma_start(out=wq_s, in_=wq_d.rearrange("p (c s) -> p c s", c=KI))
    nc.scalar.dma_start(out=wk_s, in_=wk_d.rearrange("p (c s) -> p c s", c=KI))
    xT_dv = xT_d.rearrange("p (c s) -> p c s", c=KI)
    for i in range(KI):
        nc.sync.dma_start(out=xT_s[:, i, :], in_=xT_dv[:, i, :])
    nc.scalar.dma_start(out=wv_s, in_=wv_d.rearrange("p (c s) -> p c s", c=KI))
    nc.scalar.dma_start(out=wo_s, in_=wo_d.rearrange("p (c s) -> p c s", c=2))

    # ---- constants for the PE-side causal mask ----
    # spv[k,q] += sum_c L[c,k] * negdiag[c,q] = -1e9 * (k > q)
    nc.vector.memset(triL, 1.0)
    nc.gpsimd.affine_select(  # keep 1 where k >= c+1, else 0
        out=triL, in_=triL, compare_op=mybir.AluOpType.is_ge,
        fill=0.0, base=-1, pattern=[[1, 128]], channel_multiplier=-1,
    )
    nc.vector.memset(negdiag, -1e9)
    nc.gpsimd.affine_select(  # keep where k >= c
        out=negdiag, in_=negdiag, compare_op=mybir.AluOpType.is_ge,
        fill=0.0, base=0, pattern=[[1, 128]], channel_multiplier=-1,
    )
    nc.gpsimd.affine_select(  # keep where k <= c  -> only diagonal survives
        out=negdiag, in_=negdiag, compare_op=mybir.AluOpType.is_ge,
        fill=0.0, base=0, pattern=[[-1, 128]], channel_multiplier=1,
    )
    # ones columns of V: ctx matmul rows 64..127 = replicated denominator
    nc.vector.memset(v_s[:, :, :, HD:], 1.0)

    def qk_one(pair, s4, which):
        w_s, dst = ((wq_s, qt_s), (wk_s, kt_s))[which]
        ps = qk_ps.tile([128, 512], F32, tag="qk", name="psqk")
        for ki in range(KI):
            nc.tensor.matmul(
                ps,
                lhsT=w_s[:, ki, 128 * pair : 128 * (pair + 1)],
                rhs=xT_s[:, ki, 512 * s4 : 512 * (s4 + 1)],
                start=(ki == 0),
                stop=(ki == KI - 1),
            )
        nc.vector.tensor_copy(
            out=dst[:, pair, 512 * s4 : 512 * (s4 + 1)], in_=ps
        )

    def qk_proj(pair, s4):
        qk_one(pair, s4, 0)
        qk_one(pair, s4, 1)

    def v_proj(sb):
        ps = qk_ps.tile([128, 256], F32, tag="qk", name="psv")
        for ki in range(KI):
            nc.tensor.matmul(
                ps,
                lhsT=xT_s[:, ki, 128 * sb : 128 * (sb + 1)],
                rhs=wv_s[:, ki, :],
                start=(ki == 0),
                stop=(ki == KI - 1),
            )
        nc.vector.tensor_copy(
            out=v_s[:, sb, :, 0:HD],
            in_=ps.rearrange("p (h d) -> p h d", h=H_LOC),
        )

    # pair-0 Q/K and V interleaved per seq chunk: attention can begin after
    # the first chunk of each is done
    for s4 in range(NQ):
        qk_proj(0, s4)
        for sb in range(4 * s4, 4 * s4 + 4):
            v_proj(sb)

    def outproj(m, s4, copy_eng="dve"):
        """Full out-proj tile (both pair chunks) -> bf16 -> DRAM."""
        op = qk_ps.tile([128, 512], F32, tag="qk", name="psop")
        for c in range(2):
            nc.tensor.matmul(
                op,
                lhsT=wo_s[:, c, 128 * m : 128 * (m + 1)],
                rhs=ctxT_s[:, c, 512 * s4 : 512 * (s4 + 1)],
                start=(c == 0),
                stop=(c == 1),
            )
        ob = ob_pool.tile([128, 512], BF16, tag="ob")
        if copy_eng == "both" and m % 2 == 1:
            nc.scalar.copy(out=ob, in_=op)
        else:
            nc.vector.tensor_copy(out=ob, in_=op)
        nc.sync.dma_start(
            out=outT_d[128 * m : 128 * (m + 1), 512 * s4 : 512 * (s4 + 1)],
            in_=ob,
        )

    for pair in range(2):
        # ---- attention for this pair (q chunks of 512, transposed) ----
        for jj in range(NQ):
            q0 = 512 * jj
            nkb = 4 * (jj + 1)
            cp = ctx_ps.tile([128, 1024], F32, tag="ctx", name="cp")
            cpv = _pair_view(cp)  # [65, 2, 512]
            def ctx_mms(kb, ptv, off):
                for i in range(2):
                    h = 2 * pair + i
                    nc.tensor.matmul(
                        cpv[:, i, off:512],
                        lhsT=v_s[:, kb, h, :],
                        rhs=ptv[:, i, off:512],
                        start=(kb == 0),
                        stop=(kb == nkb - 1),
                    )  # rows 0-63: ctx; rows 64-127: D replicated

            pending = None  # (kb, ptv, off): ctx lags one kb behind exp
            for kb in range(nkb):
                d = kb - 4 * jj
                off = max(0, 128 * d)
                sp = sc_ps.tile([128, 1024], F32, tag="sc", name="sp")
                spv = _pair_view(sp)
                # paired scores matmuls (row groups 0-1 / 2-3 concurrent)
                for i in range(2):
                    nc.tensor.matmul(
                        spv[:, i, off:512],
                        lhsT=kt_s[
                            64 * i : 64 * i + 64, pair, 128 * kb : 128 * (kb + 1)
                        ],
                        rhs=qt_s[64 * i : 64 * i + 64, pair, q0 + off : q0 + 512],
                        start=True,
                        stop=True,
                    )
                if d >= 0:  # diagonal block: accumulate -1e9 upper triangle
                    for i in range(2):
                        nc.tensor.matmul(
                            spv[:, i, off : off + 128],
                            lhsT=triL,
                            rhs=negdiag,
                            start=False,
                            stop=True,
                            skip_group_check=True,
                        )
                pt = pt_pool.tile([128, 1024], BF16, tag="pt")
                ptv = _pair_view(pt)
                nc.scalar.activation(
                    out=ptv[:, :, off:512],
                    in_=spv[:, :, off:512],
                    func=EXP,
                    scale=float(SCALE),
                )
                if pending is not None:
                    ctx_mms(*pending)
                pending = (kb, ptv, off)
                if pair == 0 and jj >= 1 and kb in (1, 3):
                    # pair-1 Q/K projection spread through pair-0 attention
                    qk_one(1, jj, kb // 2)
                if pair == 1 and jj >= 1 and kb < 8:
                    # out-proj of the previous seq chunk, spread across kbs
                    outproj(kb, jj - 1)
            ctx_mms(*pending)
            # stage psum -> sbuf (releases cp); rows 0-63 ctx, rows 64-127 =
            # D broadcast. D staged to base partition 0: custom-DVE ops
            # (reciprocal_approx_fast) only work at partition offset 0.
            stage = r_pool.tile([64, 1024], F32, tag="stage")
            nc.vector.tensor_copy(out=stage, in_=cp[0:HD, :])
            std = r_pool.tile([64, 1024], F32, tag="std")
            nc.vector.tensor_copy(out=std, in_=cp[HD:, :])
            stv = stage.rearrange("p (h q) -> p h q", h=2)
            r64 = r64_pool.tile([64, 1024], F32, tag="r64")
            nc.vector.reciprocal_approx_fast(out=r64, in_=std)
            for i in range(2):
                nc.vector.tensor_mul(
                    out=ctxT_s[64 * i : 64 * i + 64, pair, q0 : q0 + 512],
                    in0=stv[0:HD, i, :],
                    in1=r64[:, 512 * i : 512 * (i + 1)],
                )
            if pair == 0 and jj == 0:
                qk_one(1, 0, 0)
                qk_one(1, 0, 1)

    # ---- tail: out-proj of the last seq chunk ----
    for m in range(8):
        outproj(m, NQ - 1, copy_eng="both")

    ctx.close()


_CACHED_NC = None


def _get_nc():
    global _CACHED_NC
    if _CACHED_NC is not None:
        return _CACHED_NC
    nc = bacc.Bacc(
        "TRN2", target_bir_lowering=False, debug=False, num_devices=N_CORES
    )
    xT_d = nc.dram_tensor("xT", [128, KI * S], BF16, kind="ExternalInput").ap()
    wq_d = nc.dram_tensor("wq", [128, KI * DH], BF16, kind="ExternalInput").ap()
    wk_d = nc.dram_tensor("wk", [128, KI * DH], BF16, kind="ExternalInput").ap()
    wv_d = nc.dram_tensor("wv", [128, KI * DH], BF16, kind="ExternalInput").ap()
    wo_d = nc.dram_tensor("wo", [128, 2 * D_OUT], BF16, kind="ExternalInput").ap()
    outT_d = nc.dram_tensor("outT", [D_OUT, S], BF16, kind="ExternalOutput").ap()
    with tile.TileContext(nc) as tc:
        _build_body(nc, tc, xT_d, wq_d, wk_d, wv_d, wo_d, outT_d)
    nc.compile()
    _CACHED_NC = nc
    return nc


def _chunked(a):
    """[C*128, N] -> [128, C*N] (partition-major chunks, on-chip layout)."""
    c = a.shape[0] // 128
    return np.ascontiguousarray(
        a.reshape(c, 128, a.shape[1]).transpose(1, 0, 2).reshape(128, -1)
    )


def _make_in_maps(x, W_q, W_k, W_v, W_o):
    bf = ml_dtypes.bfloat16
    in_maps = []
    xT = [_chunked(np.ascontiguousarray(x[b].T)).astype(bf) for b in range(B)]
    for c in range(N_CORES):
        b, g = c // 4, c % 4
        sl = slice(DH * g, DH * (g + 1))
        in_maps.append(
            {
                "xT": xT[b],
                "wq": _chunked(np.ascontiguousarray(W_q[:, sl])).astype(bf),
                "wk": _chunked(np.ascontiguousarray(W_k[:, sl])).astype(bf),
                "wv": _chunked(np.ascontiguousarray(W_v[:, sl])).astype(bf),
                "wo": _chunked(np.ascontiguousarray(W_o[sl, :])).astype(bf),
            }
        )
    return in_maps


def run_cores(x, W_q, W_k, W_v, W_o, **spmd_kwargs):
    """Compile (cached), run on 8 cores, return raw results object."""
    nc = _get_nc()
    in_maps = _make_in_maps(x, W_q, W_k, W_v, W_o)
    return run_bass_kernel_spmd(
        nc, in_maps, core_ids=list(range(N_CORES)), **spmd_kwargs
    )


def gather(results, b_o):
    out = np.empty((B, S, D_OUT), np.float32)
    for b in range(B):
        acc = results[4 * b]["outT"].astype(np.float32).copy()
        for g in range(1, 4):
            acc += results[4 * b + g]["outT"]
        out[b] = acc.T + b_o.astype(np.float32)[None, :]
    return out


def kernel(x, W_q, W_k, W_v, W_o, b_o):
    x = np.asarray(x)
    res = run_cores(
        x, np.asarray(W_q), np.asarray(W_k), np.asarray(W_v), np.asarray(W_o)
    )
    return gather(res.results, np.asarray(b_o))


# revision 24
# speedup vs baseline: 1.2392x; 1.0666x over previous
"""Multi-head self-attention (causal) Trainium2 Bass kernel, 8-core SPMD.

Problem: B=2, S=2048, D_IN=1024, D_OUT=1024, H=16 heads (hd=64), causal
softmax, out-proj with bias.

Sharding: core c -> (batch b = c // 4, head-group g = c % 4). Each core
computes 4 heads of one batch: data-parallel over b, tensor-parallel over
heads (W_q/W_k/W_v column shards, W_o row shard). Host sums the 4 partial
out-proj results per batch and adds b_o.

On-core layout is fully transposed (feature-major):
  xT   [D_IN, S]                (host pre-transposes x[b])
  Qt,Kt[hd*4, S]  = W^T @ xT    (pair-major: 2 sbuf tiles of [128, S])
  V    [S, hd*4]  (normal orientation, ones column appended per head)
  St   [k, q] scores transposed; Pt = exp(St/8) in bf16
  ctxT [hd*4, S] with softmax denominator from the ones column
  outT [D_OUT, S] partial = Wo_shard^T @ ctxT  (host transposes back)

All matmul operands bf16 (fp32 PSUM accumulate); softmax in fp32.
Scores for a head pair are packed: head0 on PE row-groups 0-1, head1 on
2-3 (concurrent matmuls), psum tiles merged so one ACT exp call covers
both heads of a k-block.
"""

import numpy as np
import ml_dtypes

import concourse.bass as bass
import concourse.bacc as bacc
import concourse.tile as tile
import concourse.mybir as mybir
from concourse.bass_utils import run_bass_kernel_spmd

N_CORES = 8
B, S, D_IN, D_OUT, H = 2, 2048, 1024, 1024, 16
H_LOC = 4  # heads per core
HD = 64
DH = H_LOC * HD  # 256 = d_out shard per core
KI = D_IN // 128  # 8 contraction chunks
NQ = S // 512  # 4 seq chunks of 512
NB = S // 128  # 16 seq blocks of 128
SCALE = 1.0 / np.sqrt(np.float32(HD))  # 0.125

BF16 = mybir.dt.bfloat16
F32 = mybir.dt.float32
EXP = mybir.ActivationFunctionType.Exp


def _pair_view(ap2d):
    """[128, 1024] tile -> [128, 2, 512] (head-major) view."""
    return ap2d.rearrange("p (h q) -> p h q", h=2)


def _build_body(nc, tc, xT_d, wq_d, wk_d, wv_d, wo_d, outT_d):
    from contextlib import ExitStack

    ctx = ExitStack()
    const = ctx.enter_context(tc.tile_pool(name="const", bufs=1))
    # PSUM: sc [128,1024]x2 = 4 banks; ctx [128,1024]x1 = 2; qk [128,512]x2 = 2
    sc_ps = ctx.enter_context(tc.tile_pool(name="sc_ps", bufs=2, space="PSUM"))
    ctx_ps = ctx.enter_context(tc.tile_pool(name="ctx_ps", bufs=1, space="PSUM"))
    qk_ps = ctx.enter_context(tc.tile_pool(name="qk_ps", bufs=2, space="PSUM"))
    pt_pool = ctx.enter_context(tc.tile_pool(name="pt", bufs=6))
    ob_pool = ctx.enter_context(tc.tile_pool(name="ob", bufs=3))
    r_pool = ctx.enter_context(tc.tile_pool(name="r", bufs=4))
    r64_pool = ctx.enter_context(tc.tile_pool(name="r64", bufs=4))

    # ---- resident SBUF tensors ----
    xT_s = const.tile([128, KI, S], BF16)
    wq_s = const.tile([128, KI, DH], BF16)
    wk_s = const.tile([128, KI, DH], BF16)
    wv_s = const.tile([128, KI, DH], BF16)
    wo_s = const.tile([128, 2, D_OUT], BF16)
    qt_s = const.tile([128, 2, S], BF16)  # [64*sub + hd, pair, q]
    kt_s = const.tile([128, 2, S], BF16)
    v_s = const.tile([128, NB, H_LOC, 2 * HD], BF16)  # cols HD.. = ones x64
    ctxT_s = const.tile([128, 2, S], BF16)
    triL = const.tile([128, 128], BF16)  # L[c,k] = 1 iff k > c
    negdiag = const.tile([128, 128], BF16)  # -1e9 * I

    # ---- input DMAs (host pre-arranged to on-chip layout; contiguous) ----
    nc.scalar.dma_start(out=wq_s, in_=wq_d.rearrange("p (c s) -> p c s", c=KI))
    nc.scalar.dma_start(out=wk_s, in_=wk_d.rearrange("p (c s) -> p c s", c=KI))
    xT_dv = xT_d.rearrange("p (c s) -> p c s", c=KI)
    for i in range(KI):
        nc.sync.dma_start(out=xT_s[:, i, :], in_=xT_dv[:, i, :])
    nc.scalar.dma_start(out=wv_s, in_=wv_d.rearrange("p (c s) -> p c s", c=KI))
    nc.scalar.dma_start(out=wo_s, in_=wo_d.rearrange("p (c s) -> p c s", c=2))

    # ---- constants for the PE-side causal mask ----
    # spv[k,q] += sum_c L[c,k] * negdiag[c,q] = -1e9 * (k > q)
    nc.vector.memset(triL, 1.0)
    nc.gpsimd.affine_select(  # keep 1 where k >= c+1, else 0
        out=triL, in_=triL, compare_op=mybir.AluOpType.is_ge,
        fill=0.0, base=-1, pattern=[[1, 128]], channel_multiplier=-1,
    )
    nc.vector.memset(negdiag, -1e9)
    nc.gpsimd.affine_select(  # keep where k >= c
        out=negdiag, in_=negdiag, compare_op=mybir.AluOpType.is_ge,
        fill=0.0, base=0, pattern=[[1, 128]], channel_multiplier=-1,
    )
    nc.gpsimd.affine_select(  # keep where k <= c  -> only diagonal survives
        out=negdiag, in_=negdiag, compare_op=mybir.AluOpType.is_ge,
        fill=0.0, base=0, pattern=[[-1, 128]], channel_multiplier=1,
    )
    # ones columns of V: ctx matmul rows 64..127 = replicated denominator
    nc.vector.memset(v_s[:, :, :, HD:], 1.0)

    def qk_one(pair, s4, which):
        w_s, dst = ((wq_s, qt_s), (wk_s, kt_s))[which]
        ps = qk_ps.tile([128, 512], F32, tag="qk", name="psqk")
        for ki in range(KI):
            nc.tensor.matmul(
                ps,
                lhsT=w_s[:, ki, 128 * pair : 128 * (pair + 1)],
                rhs=xT_s[:, ki, 512 * s4 : 512 * (s4 + 1)],
                start=(ki == 0),
                stop=(ki == KI - 1),
            )
        nc.vector.tensor_copy(
            out=dst[:, pair, 512 * s4 : 512 * (s4 + 1)], in_=ps
        )

    def qk_proj(pair, s4):
        qk_one(pair, s4, 0)
        qk_one(pair, s4, 1)

    def v_proj(sb):
        ps = qk_ps.tile([128, 256], F32, tag="qk", name="psv")
        for ki in range(KI):
            nc.tensor.matmul(
                ps,
                lhsT=xT_s[:, ki, 128 * sb : 128 * (sb + 1)],
                rhs=wv_s[:, ki, :],
                start=(ki == 0),
                stop=(ki == KI - 1),
            )
        nc.vector.tensor_copy(
            out=v_s[:, sb, :, 0:HD],
            in_=ps.rearrange("p (h d) -> p h d", h=H_LOC),
        )

    # ---- PE warm-up: junk matmuls engage the HAM clock gate (K=8/8)
    # while the input DMAs stream in; results are never read ----
    wu = const.tile([128, 512], BF16)
    nc.vector.memset(wu, 0.0)
    wp = qk_ps.tile([128, 512], F32, tag="qk", name="wp")
    for _ in range(32):
        nc.tensor.matmul(wp, lhsT=wu[:, 0:128], rhs=wu, start=True, stop=True)

    # pair-0 Q/K plus the first V blocks: the minimum to start attention.
    # Remaining V blocks are interleaved into pair-0 attention below.
    for s4 in range(NQ):
        qk_proj(0, s4)
    for sb in range(4):
        v_proj(sb)

    def outproj(m, s4, copy_eng="dve"):
        """Full out-proj tile (both pair chunks) -> bf16 -> DRAM."""
        op = qk_ps.tile([128, 512], F32, tag="qk", name="psop")
        for c in range(2):
            nc.tensor.matmul(
                op,
                lhsT=wo_s[:, c, 128 * m : 128 * (m + 1)],
                rhs=ctxT_s[:, c, 512 * s4 : 512 * (s4 + 1)],
                start=(c == 0),
                stop=(c == 1),
            )
        ob = ob_pool.tile([128, 512], BF16, tag="ob")
        if copy_eng == "both" and m % 2 == 1:
            nc.scalar.copy(out=ob, in_=op)
        else:
            nc.vector.tensor_copy(out=ob, in_=op)
        nc.sync.dma_start(
            out=outT_d[128 * m : 128 * (m + 1), 512 * s4 : 512 * (s4 + 1)],
            in_=ob,
        )

    # ---- attention: flat per-pair pipeline, ctx matmuls lag two k-blocks
    # behind exp so the in-order PE stream never blocks on the softmax ----
    for pair in range(2):
        cps = {}

        def ctx_mms(jj, kb, ptv, off):
            cpv, nkb = cps[jj]
            for i in range(2):
                h = 2 * pair + i
                nc.tensor.matmul(
                    cpv[:, i, off:512],
                    lhsT=v_s[:, kb, h, :],
                    rhs=ptv[:, i, off:512],
                    start=(kb == 0),
                    stop=(kb == nkb - 1),
                )  # rows 0-63: ctx; rows 64-127: D replicated

        def normalize(jj):
            done_norms.add(jj)
            cpv, _ = cps.pop(jj)
            cp2 = cpv.rearrange("p h q -> p (h q)")
            q0 = 512 * jj
            # D staged to base partition 0: custom-DVE ops only work at
            # partition offset 0
            stage = r_pool.tile([64, 1024], F32, tag="stage")
            nc.vector.tensor_copy(out=stage, in_=cp2[0:HD, :])
            std = r_pool.tile([64, 1024], F32, tag="std")
            nc.vector.tensor_copy(out=std, in_=cp2[HD:, :])
            stv = stage.rearrange("p (h q) -> p h q", h=2)
            r64 = r64_pool.tile([64, 1024], F32, tag="r64")
            nc.vector.reciprocal_approx_fast(out=r64, in_=std)
            for i in range(2):
                nc.vector.tensor_mul(
                    out=ctxT_s[64 * i : 64 * i + 64, pair, q0 : q0 + 512],
                    in0=stv[0:HD, i, :],
                    in1=r64[:, 512 * i : 512 * (i + 1)],
                )

        units = []  # (jj, kb)
        for jj in range(NQ):
            units += [(jj, kb) for kb in range(4 * (jj + 1))]
        fillers = []  # (kind, args) interleave work, one per unit slot
        if pair == 0:
            for sb in range(4, NB):
                fillers.append(("v", sb))
            for s4 in range(NQ):
                fillers.append(("qk", s4, 0))
                fillers.append(("qk", s4, 1))
        else:
            for s4 in range(NQ - 1):
                for m in range(8):
                    fillers.append(("op", m, s4))

        pending = []  # ctx units not yet emitted
        done_norms = set()
        nfill = 0
        for u, (jj, kb) in enumerate(units):
            if kb == 0:
                cp = ctx_ps.tile([128, 1024], F32, tag="ctx", name="cp")
                cps[jj] = (_pair_view(cp), 4 * (jj + 1))
            q0 = 512 * jj
            d = kb - 4 * jj
            off = max(0, 128 * d)
            sp = sc_ps.tile([128, 1024], F32, tag="sc", name="sp")
            spv = _pair_view(sp)
            # paired scores matmuls (row groups 0-1 / 2-3 concurrent)
            for i in range(2):
                nc.tensor.matmul(
                    spv[:, i, off:512],
                    lhsT=kt_s[64 * i : 64 * i + 64, pair, 128 * kb : 128 * (kb + 1)],
                    rhs=qt_s[64 * i : 64 * i + 64, pair, q0 + off : q0 + 512],
                    start=True,
                    stop=True,
                )
            if d >= 0:  # diagonal block: accumulate -1e9 upper triangle
                for i in range(2):
                    nc.tensor.matmul(
                        spv[:, i, off : off + 128],
                        lhsT=triL,
                        rhs=negdiag,
                        start=False,
                        stop=True,
                        skip_group_check=True,
                    )
            pt = pt_pool.tile([128, 1024], BF16, tag="pt")
            ptv = _pair_view(pt)
            nc.scalar.activation(
                out=ptv[:, :, off:512],
                in_=spv[:, :, off:512],
                func=EXP,
                scale=float(SCALE),
            )
            pending.append((jj, kb, ptv, off))
            # interleave filler work roughly every other unit; out-proj
            # fillers must wait until their seq chunk has been normalized
            if nfill < len(fillers) and u % 2 == 1:
                f = fillers[nfill]
                if f[0] != "op" or f[2] in done_norms:
                    nfill += 1
                    if f[0] == "v":
                        v_proj(f[1])
                    elif f[0] == "qk":
                        qk_one(1, f[1], f[2])
                    else:
                        outproj(f[1], f[2])
            # retire ctx lag-2; finish a chunk fully once its last exp is in
            while len(pending) > 2 or (pending and pending[0][0] < jj):
                pj, pk, pptv, poff = pending.pop(0)
                ctx_mms(pj, pk, pptv, poff)
                if pk == 4 * (pj + 1) - 1:
                    normalize(pj)
        for pj, pk, pptv, poff in pending:
            ctx_mms(pj, pk, pptv, poff)
            if pk == 4 * (pj + 1) - 1:
                normalize(pj)
        while nfill < len(fillers):  # drain any fillers that didn't fit
            f = fillers[nfill]
            nfill += 1
            if f[0] == "v":
                v_proj(f[1])
            elif f[0] == "qk":
                qk_one(1, f[1], f[2])
            else:
                outproj(f[1], f[2])

    # ---- tail: out-proj of the last seq chunk ----
    for m in range(8):
        outproj(m, NQ - 1, copy_eng="both")

    ctx.close()


_CACHED_NC = None


def _get_nc():
    global _CACHED_NC
    if _CACHED_NC is not None:
        return _CACHED_NC
    nc = bacc.Bacc(
        "TRN2", target_bir_lowering=False, debug=False, num_devices=N_CORES
    )
    xT_d = nc.dram_tensor("xT", [128, KI * S], BF16, kind="ExternalInput").ap()
    wq_d = nc.dram_tensor("wq", [128, KI * DH], BF16, kind="ExternalInput").ap()
    wk_d = nc.dram_tensor("wk", [128, KI * DH], BF16, kind="ExternalInput").ap()
    wv_d = nc.dram_tensor("wv", [128, KI * DH], BF16, kind="ExternalInput").ap()
    wo_d = nc.dram_tensor("wo", [128, 2 * D_OUT], BF16, kind="ExternalInput").ap()
    outT_d = nc.dram_tensor("outT", [D_OUT, S], BF16, kind="ExternalOutput").ap()
    with tile.TileContext(nc) as tc:
        _build_body(nc, tc, xT_d, wq_d, wk_d, wv_d, wo_d, outT_d)
    nc.compile()
    _CACHED_NC = nc
    return nc


def _chunked(a):
    """[C*128, N] -> [128, C*N] (partition-major chunks, on-chip layout)."""
    c = a.shape[0] // 128
    return np.ascontiguousarray(
        a.reshape(c, 128, a.shape[1]).transpose(1, 0, 2).reshape(128, -1)
    )


def _make_in_maps(x, W_q, W_k, W_v, W_o):
    bf = ml_dtypes.bfloat16
    in_maps = []
    xT = [_chunked(np.ascontiguousarray(x[b].T)).astype(bf) for b in range(B)]
    for c in range(N_CORES):
        b, g = c // 4, c % 4
        sl = slice(DH * g, DH * (g + 1))
        in_maps.append(
            {
                "xT": xT[b],
                "wq": _chunked(np.ascontiguousarray(W_q[:, sl])).astype(bf),
                "wk": _chunked(np.ascontiguousarray(W_k[:, sl])).astype(bf),
                "wv": _chunked(np.ascontiguousarray(W_v[:, sl])).astype(bf),
                "wo": _chunked(np.ascontiguousarray(W_o[sl, :])).astype(bf),
            }
        )
    return in_maps


def run_cores(x, W_q, W_k, W_v, W_o, **spmd_kwargs):
    """Compile (cached), run on 8 cores, return raw results object."""
    nc = _get_nc()
    in_maps = _make_in_maps(x, W_q, W_k, W_v, W_o)
    return run_bass_kernel_spmd(
        nc, in_maps, core_ids=list(range(N_CORES)), **spmd_kwargs
    )


def gather(results, b_o):
    out = np.empty((B, S, D_OUT), np.float32)
    for b in range(B):
        acc = results[4 * b]["outT"].astype(np.float32).copy()
        for g in range(1, 4):
            acc += results[4 * b + g]["outT"]
        out[b] = acc.T + b_o.astype(np.float32)[None, :]
    return out


def kernel(x, W_q, W_k, W_v, W_o, b_o):
    x = np.asarray(x)
    res = run_cores(
        x, np.asarray(W_q), np.asarray(W_k), np.asarray(W_v), np.asarray(W_o)
    )
    return gather(res.results, np.asarray(b_o))
